# revision 12
# baseline (speedup 1.0000x reference)
"""Trainium2 Bass kernel for nn_AttFKANBlock (2-layer Fourier-KAN block + CBAM tail).

Strategy: pure data-parallel over batch B=2048 across 8 NeuronCores (256 rows/core,
all parameters replicated). The whole computation is independent per batch row, so
no collectives are needed.

Per core, each FKAN layer out[b,o] = sum_{i,g} cos/sin((g+1)*h[b,i]) * coef[c,o,i,g]
is one dense GEMM with contraction K = 2*G*D (+1 bias chunk), in mixed precision:
  - harmonic g=1 (92% of output variance, coef ~ 1/g^2) runs in bf16,
  - harmonics g=2..8 run in fp8 e4m3 with MatmulPerfMode.DoubleRow (2 K-subtiles
    per instruction, 2x PE throughput, weights at 1 byte -> ~half the HBM traffic),
  - all weights are pre-scaled by 2^11 on the host so the fp8 blocks sit in e4m3's
    normal range; the common scale keeps every chunk in ONE psum accumulation group
    per (batch-tile, out-chunk) and is folded out at drain via act scale=2^-11.
Fourier features sin/cos(2pi*t) use fp32 magic-number range reduction (+/-1.5*2^23)
on the Vector engine and the ACT Sin LUT, batched over 8 input chunks per
instruction to amortize the ACT 352ns fixed overhead; fp8 feature tiles are written
directly by ACT as [128, ic, 2(cos/sin), 256b] so a DoubleRow lhsT slice is free.
The CBAM tail (channel-attention MLP + spatial attention) runs in [o-part, b-free]
layout; spatial attention's 7x7 conv on 1x1 spatial input reduces to its center tap.
"""
import numpy as np
import ml_dtypes
from contextlib import ExitStack

import concourse.tile as tile
from concourse import bacc, mybir
from concourse.bass import ts, ds
from concourse.bass_utils import run_bass_kernel_spmd
from concourse.masks import make_identity

f32 = mybir.dt.float32
bf16 = mybir.dt.bfloat16
fp8 = mybir.dt.float8e4
AF = mybir.ActivationFunctionType
OP = mybir.AluOpType
DR = mybir.MatmulPerfMode.DoubleRow

P = 128
PI = float(np.pi)
TWO_PI = float(2.0 * np.pi)
MAGIC = float(1.5 * 2**23)  # fp32 round-to-nearest-integer magic constant
WSC = float(2.0**11)        # weight pre-scale (host side), folded out at drain
FP8_CLIP = 240.0            # TRN e4m3 max normal

FULL_CFG = dict(D=2048, G=8, BPC=256, HID=128, EPS=1e-5)
N_CORES = 8


def build_program(D=2048, G=8, BPC=256, HID=128, EPS=1e-5, w_bufs=4, fs_bufs=4,
                  reps=1, diag='none'):
    DC = D // P              # D chunks (16)
    NBT = BPC // P           # batch tiles per core (2)
    QB = DC // 4             # ic chunks per feature batch (4)
    NQ = DC // QB            # feature batches per harmonic (4)
    NG8 = G - 1              # fp8 harmonics (2..G)
    NPAIR = NG8 * DC         # fp8 weight pairs per layer (112)
    NBCH = 2 * DC + 1        # bf16 chunks per layer incl. bias (33)
    OCW = min(512, D)        # psum free width
    OC = D // OCW            # o-chunks (4)
    assert OC * NBT <= 8 and HID == P and D % OCW == 0
    BNW = min(512, D)        # bn_stats chunk width
    BNC = D // BNW

    nc = bacc.Bacc("TRN2", target_bir_lowering=False, debug=False, num_devices=N_CORES)

    x_d = nc.dram_tensor("x", [BPC, D], f32, kind="ExternalInput")
    wb_ds = [nc.dram_tensor(f"wb{l}", [NBCH * P, D], bf16, kind="ExternalInput")
             for l in range(2)]
    wf_ds = [nc.dram_tensor(f"wf{l}", [NPAIR * P, 2 * D], fp8, kind="ExternalInput")
             for l in range(2)]
    lnw_d = nc.dram_tensor("lnw", [2, D], f32, kind="ExternalInput")
    lnb_d = nc.dram_tensor("lnb", [2, D], f32, kind="ExternalInput")
    mw1_d = nc.dram_tensor("mw1", [D, HID], bf16, kind="ExternalInput")   # ca_w1.T
    mw2_d = nc.dram_tensor("mw2", [HID, D], bf16, kind="ExternalInput")   # ca_w2.T
    mb1_d = nc.dram_tensor("mb1", [HID], f32, kind="ExternalInput")       # ca_b1
    mb2_d = nc.dram_tensor("mb2", [D], f32, kind="ExternalInput")         # ca_b2
    sa3_d = nc.dram_tensor("sa3", [3], f32, kind="ExternalInput")         # w00, w01, sa_b
    out_d = nc.dram_tensor("out", [BPC, D], f32, kind="ExternalOutput")

    with tile.TileContext(nc) as tc, ExitStack() as ctx:
        pool = lambda name, bufs, **kw: ctx.enter_context(tc.tile_pool(name=name, bufs=bufs, **kw))
        p_const = pool("const", 1)
        p_x = pool("x", NBT + 1)
        p_r = pool("r", NBT)
        p_big = pool("big", 3)
        p_zt = pool("zt", 1)
        p_f8 = pool("f8", 2)      # fp8 feature tiles [P, QB, 2, BPC]
        p_fb = pool("fb", 2)      # bf16 feature tiles [P, QB, 2, BPC]
        p_fs = pool("fs", fs_bufs)  # range-reduction temps [P, QB, BPC] f32
        p_wb = pool("wb", w_bufs)
        p_wf = pool("wf", w_bufs + 4)
        p_vt = pool("vt", DC)
        p_vtb = pool("vtb", DC)
        p_ca = pool("ca", 2)
        p_sm = pool("sm", 2)
        p_ps = pool("ps", 8, space="PSUM")
        p_dram = pool("dscratch", 1, space="DRAM")

        _psn = [0]

        def psum_tile():
            _psn[0] += 1
            return p_ps.tile([P, OCW], f32, tag="ps", name=f"ps{_psn[0]}")

        # ---- constants / parameter staging ----
        ident = p_const.tile([P, P], f32)
        make_identity(nc, ident)
        featx = p_const.tile([P, BPC], bf16)   # bias feature chunk: row0 = 1, rest 0
        nc.vector.memset(featx[:], 0.0)
        nc.vector.memset(featx[0:1, :], 1.0)
        halfpi = p_const.tile([P, 1], f32)
        nc.vector.memset(halfpi[:], PI / 2)
        epsb = p_const.tile([P, 1], f32)
        nc.vector.memset(epsb[:], float(EPS))
        ones_col = p_const.tile([P, 1], f32)
        nc.vector.memset(ones_col[:], 1.0)

        lnw_sb = p_const.tile([P, 2, DC], f32)
        lnb_sb = p_const.tile([P, 2, DC], f32)
        with nc.allow_non_contiguous_dma(reason="small param staging"):
            nc.sync.dma_start(lnw_sb[:], lnw_d.ap().rearrange("l (c p) -> p l c", p=P))
            nc.sync.dma_start(lnb_sb[:], lnb_d.ap().rearrange("l (c p) -> p l c", p=P))
        # scale by 1/(2pi) so zt holds (z*w + b)/(2pi)
        nc.vector.tensor_scalar_mul(lnw_sb[:], lnw_sb[:], 1.0 / TWO_PI)
        nc.vector.tensor_scalar_mul(lnb_sb[:], lnb_sb[:], 1.0 / TWO_PI)

        mw1_sb = p_const.tile([P, DC, HID], bf16)
        nc.sync.dma_start(mw1_sb[:], mw1_d.ap().rearrange("(c p) h -> p c h", p=P))
        mw2_sb = p_const.tile([P, D], bf16)
        nc.sync.dma_start(mw2_sb[:], mw2_d.ap())
        mb1_sb = p_const.tile([P, 1], f32)
        with nc.allow_non_contiguous_dma(reason="small param staging"):
            nc.sync.dma_start(mb1_sb[:], mb1_d.ap().rearrange("(p a) -> p a", a=1))
            mb2x2 = p_const.tile([P, DC], f32)
            nc.sync.dma_start(mb2x2[:], mb2_d.ap().rearrange("(c p) -> p c", p=P))
        nc.vector.tensor_scalar_mul(mb2x2[:], mb2x2[:], 2.0)
        sa3_sb = p_const.tile([1, 3], f32)
        with nc.allow_non_contiguous_dma(reason="small param staging"):
            nc.sync.dma_start(sa3_sb[:], sa3_d.ap().rearrange("(a k) -> a k", a=1))

        def kernel_body():
            # ---- load x ----
            xt = []
            for bt in range(NBT):
                t = p_x.tile([P, D], f32)
                nc.scalar.dma_start(t[:], x_d.ap()[ts(bt, P), :])
                xt.append(t)

            # ---- helpers ----
            def ln_modify(src, dst):
                """dst = (src - mean)/sqrt(var+eps), rows of [P, D]. dst may alias src."""
                stats = p_sm.tile([P, BNC, 6], f32, tag="bnstats")
                for c in range(BNC):
                    nc.vector.bn_stats(stats[:, c], src[:, ts(c, BNW)])
                aggr = p_sm.tile([P, 2], f32, tag="bnaggr")
                nc.vector.bn_aggr(aggr[:], stats[:].rearrange("p a b -> p (a b)"))
                sd = p_sm.tile([P, 1], f32, tag="sd")
                nc.scalar.activation(sd[:], aggr[:, 1:2], AF.Sqrt, bias=epsb[:])
                rstd = p_sm.tile([P, 1], f32, tag="rstd")
                nc.vector.reciprocal(rstd[:], sd[:])
                nc.vector.tensor_scalar(dst[:], src[:], aggr[:, 0:1], rstd[:],
                                        OP.subtract, OP.mult)

            def transpose_zt(z_tiles, l):
                """z [b-part, D] tiles -> zt [P, DC, BPC] with (z*w+b)/(2pi) applied."""
                zt = p_zt.tile([P, DC, BPC], f32)
                for bt in range(NBT):
                    for c in range(DC):
                        pst = psum_tile()
                        nc.tensor.transpose(pst[:, :P], z_tiles[bt][:, ts(c, P)], ident)
                        nc.scalar.activation(zt[:, c, ds(bt * P, P)], pst[:, :P], AF.Identity,
                                             bias=lnb_sb[:, l, c:c + 1],
                                             scale=lnw_sb[:, l, c:c + 1])
                return zt

            def features_quarter(zt, g, q, out_dt, fpool):
                """Range-reduce zt[:, q*QB:(q+1)*QB, :] * g and emit a feature tile
                [P, QB, 2, BPC] (dim2: 0=cos, 1=sin) in out_dt."""
                X = zt[:, ds(q * QB, QB), :]
                if g != 1:
                    tt = p_fs.tile([P, QB, BPC], f32, tag="fs")
                    nc.vector.tensor_scalar_mul(tt[:], X, float(g))
                    src = tt[:]
                else:
                    src = X
                # nsp = src + MAGIC ; in-place -> dsin = (nsp - MAGIC) - src
                nsp = p_fs.tile([P, QB, BPC], f32, tag="fs")
                nc.vector.tensor_scalar_add(nsp[:], src, MAGIC)
                nc.vector.scalar_tensor_tensor(nsp[:], nsp[:], MAGIC, src,
                                               OP.subtract, OP.subtract)
                # ncp = (src + 0.25) + MAGIC ; in-place -> dcos = (ncp - MAGIC) - src
                ncp = p_fs.tile([P, QB, BPC], f32, tag="fs")
                nc.vector.tensor_scalar(ncp[:], src, 0.25, MAGIC, OP.add, OP.add)
                nc.vector.scalar_tensor_tensor(ncp[:], ncp[:], MAGIC, src,
                                               OP.subtract, OP.subtract)
                feat = fpool.tile([P, QB, 2, BPC], out_dt, tag="feat")
                nc.scalar.activation(feat[:, :, 0, :], ncp[:], AF.Sin, bias=halfpi[:],
                                     scale=-TWO_PI)
                nc.scalar.activation(feat[:, :, 1, :], nsp[:], AF.Sin, scale=-TWO_PI)
                return feat

            def fkan_matmul(wb_d, wf_d, zt):
                """Stream all weights once; accs[bt][oc] psum tiles [128b, OCW o].

                bf16 g=1 chunks (PE-heavy) are interleaved among the fp8
                DoubleRow pairs (DMA-heavy) so neither engine stalls long; the
                bias chunk goes last and carries stop=True."""
                accs = [[psum_tile() for _ in range(OC)] for _ in range(NBT)]
                first = [True]

                def emit_bf16(feat, icl, j, k):
                    wt = p_wb.tile([P, D], bf16, tag="wb")
                    if diag != 'pe':
                        nc.sync.dma_start(wt[:], wb_d.ap()[ts(k, P), :])
                    else:
                        nc.gpsimd.memset(wt[:], 0.25)
                    if diag != 'dma':
                        for bt in range(NBT):
                            lhsT = feat[:, icl, j, ts(bt, P)]
                            for oc in range(OC):
                                nc.tensor.matmul(accs[bt][oc][:], lhsT,
                                                 wt[:, ts(oc, OCW)],
                                                 start=first[0], stop=False)
                            first[0] = False

                def emit_fp8(feat, icl, pr):
                    wt = p_wf.tile([P, 2, D], fp8, tag="wf")
                    if diag != 'pe':
                        nc.sync.dma_start(wt[:], wf_d.ap()[ts(pr, P), :]
                                          .rearrange("p (j d) -> p j d", j=2))
                    else:
                        nc.gpsimd.memset(wt[:], 0.25)
                    if diag != 'dma':
                        for bt in range(NBT):
                            lhsT = feat[:, icl, :, ts(bt, P)]
                            for oc in range(OC):
                                nc.tensor.matmul(accs[bt][oc][:], lhsT,
                                                 wt[:, :, ts(oc, OCW)],
                                                 start=False, stop=False,
                                                 perf_mode=DR)

                for q in range(NQ):
                    featb = features_quarter(zt, 1, q, bf16, p_fb)
                    bq = [(icl, j) for icl in range(QB) for j in range(2)]
                    bqi = 0
                    # lead with 1 bf16 chunk (covers start=True), then 1 per harmonic
                    icl, j = bq[bqi]
                    emit_bf16(featb, icl, j, (q * QB + icl) * 2 + j)
                    bqi += 1
                    for g in range(2, G + 1):
                        feat = features_quarter(zt, g, q, fp8, p_f8)
                        for icl in range(QB):
                            emit_fp8(feat, icl, (g - 2) * DC + q * QB + icl)
                        if bqi < len(bq):
                            icl, j = bq[bqi]
                            emit_bf16(featb, icl, j, (q * QB + icl) * 2 + j)
                            bqi += 1
                    while bqi < len(bq):
                        icl, j = bq[bqi]
                        emit_bf16(featb, icl, j, (q * QB + icl) * 2 + j)
                        bqi += 1

                # bias chunk last: feature row0 = 1.0, carries stop=True
                wt = p_wb.tile([P, D], bf16, tag="wb")
                if diag != 'pe':
                    nc.sync.dma_start(wt[:], wb_d.ap()[ts(NBCH - 1, P), :])
                else:
                    nc.gpsimd.memset(wt[:], 0.25)
                if diag != 'dma':
                    for bt in range(NBT):
                        for oc in range(OC):
                            nc.tensor.matmul(accs[bt][oc][:], featx[:, ts(bt, P)],
                                             wt[:, ts(oc, OCW)], start=False, stop=True)
                return accs

            # ---- layer 1 ----
            z1 = []
            for bt in range(NBT):
                z = p_big.tile([P, D], f32, tag="big")
                ln_modify(xt[bt], z)
                z1.append(z)
            zt1 = transpose_zt(z1, 0)
            accs1 = fkan_matmul(wb_ds[0], wf_ds[0], zt1)

            # relu drain -> r (folds out the 2^11 weight scale)
            rt = []
            for bt in range(NBT):
                r = p_r.tile([P, D], f32)
                for oc in range(OC):
                    nc.scalar.activation(r[:, ts(oc, OCW)], accs1[bt][oc][:], AF.Relu,
                                         scale=1.0 / WSC)
                rt.append(r)

            # ---- layer 2 ----
            for bt in range(NBT):
                ln_modify(rt[bt], rt[bt])
            zt2 = transpose_zt(rt, 1)
            accs2 = fkan_matmul(wb_ds[1], wf_ds[1], zt2)

            # drain f2 in [b,o] layout
            f2 = []
            for bt in range(NBT):
                f = p_big.tile([P, D], f32, tag="big")
                for oc in range(OC):
                    nc.scalar.activation(f[:, ts(oc, OCW)], accs2[bt][oc][:], AF.Copy,
                                         scale=1.0 / WSC)
                f2.append(f)

            # ---- CBAM tail in [o-part, b-free] layout ----
            vt = [p_vt.tile([P, BPC], f32, tag="vt", name=f"vt{c}") for c in range(DC)]
            vtb = [p_vtb.tile([P, BPC], bf16, tag="vtb", name=f"vtb{c}") for c in range(DC)]
            for bt in range(NBT):
                for c in range(DC):
                    pst = psum_tile()
                    nc.tensor.transpose(pst[:, :P], f2[bt][:, ts(c, P)], ident)
                    nc.scalar.activation(vt[c][:, ts(bt, P)], pst[:, :P], AF.Copy)
                    nc.vector.tensor_copy(vtb[c][:, ts(bt, P)], pst[:, :P])

            # channel-attention MLP: ca = sigmoid(2*(W2 relu(W1 v + b1) + b2))
            psm = psum_tile()
            for c in range(DC):
                nc.tensor.matmul(psm[:, :BPC], mw1_sb[:, c], vtb[c][:],
                                 start=(c == 0), stop=(c == DC - 1))
            hbuf = p_sm.tile([P, BPC], bf16, tag="hbuf")
            nc.scalar.activation(hbuf[:], psm[:, :BPC], AF.Relu, bias=mb1_sb[:])
            # xc = v * ca (in place on vt), ca consumed as produced
            for c in range(DC):
                ps2 = psum_tile()
                nc.tensor.matmul(ps2[:, :BPC], mw2_sb[:, ts(c, P)], hbuf[:],
                                 start=True, stop=True)
                cac = p_ca.tile([P, BPC], bf16, tag="ca")
                nc.scalar.activation(cac[:], ps2[:, :BPC], AF.Sigmoid,
                                     bias=mb2x2[:, c:c + 1], scale=2.0)
                nc.vector.tensor_mul(vt[c][:], vt[c][:], cac[:])

            # spatial attention: sg = sigmoid(w00*mean_c + w01*max_c + sa_b)
            psmean = psum_tile()
            for c in range(DC):
                nc.tensor.matmul(psmean[:1, :BPC], ones_col[:, :1], vt[c][:],
                                 start=(c == 0), stop=(c == DC - 1))
            meanc = p_sm.tile([1, BPC], f32, tag="meanc")
            nc.vector.tensor_scalar_mul(meanc[:], psmean[:1, :BPC], 1.0 / D)
            mx = p_sm.tile([P, BPC], f32, tag="mx")
            nc.vector.tensor_copy(mx[:], vt[0][:])
            for c in range(1, DC):
                nc.vector.tensor_max(mx[:], mx[:], vt[c][:])
            # cross-partition max: PE-transpose each batch tile, reduce over free dim,
            # then round-trip through DRAM to land back in [1, BPC] row layout.
            maxd = p_dram.tile([1, BPC], f32)
            for bt in range(NBT):
                pst = psum_tile()
                nc.tensor.transpose(pst[:, :P], mx[:, ts(bt, P)], ident)
                maxb = p_sm.tile([P, 1], f32, tag="maxb")
                nc.vector.tensor_reduce(maxb[:], pst[:, :P], mybir.AxisListType.X, OP.max)
                with nc.allow_non_contiguous_dma(reason="tiny max round-trip"):
                    nc.scalar.dma_start(maxd[:, ds(bt * P, P)].rearrange("a p -> p a"), maxb[:])
            maxrow = p_sm.tile([1, BPC], f32, tag="maxrow")
            nc.scalar.dma_start(maxrow[:], maxd[:])
            t2 = p_sm.tile([1, BPC], f32, tag="t2")
            nc.vector.tensor_scalar_mul(t2[:], maxrow[:], sa3_sb[:, 1:2])
            sa_t = p_sm.tile([1, BPC], f32, tag="sat")
            nc.vector.scalar_tensor_tensor(sa_t[:], meanc[:], sa3_sb[:, 0:1], t2[:],
                                           OP.mult, OP.add)
            sg = p_sm.tile([1, BPC], f32, tag="sg")
            nc.scalar.activation(sg[:], sa_t[:], AF.Sigmoid, bias=sa3_sb[:, 2:3])

            # broadcast sg over partitions via DRAM round-trip -> [P, NBT] per-partition scalars
            sgd = p_dram.tile([1, BPC], f32)
            nc.scalar.dma_start(sgd[:], sg[:])
            sgpb = p_sm.tile([P, NBT], f32, tag="sgpb")
            with nc.allow_non_contiguous_dma(reason="tiny broadcast round-trip"):
                nc.scalar.dma_start(sgpb[:], sgd[:].rearrange("a (t p) -> p (a t)", p=P))

            # final: out = x + xc^T * sg
            for bt in range(NBT):
                outsb = p_big.tile([P, D], f32, tag="big")
                for c in range(DC):
                    pst = psum_tile()
                    nc.tensor.transpose(pst[:, :P], vt[c][:, ts(bt, P)], ident)
                    nc.vector.scalar_tensor_tensor(outsb[:, ts(c, P)], pst[:, :P],
                                                   sgpb[:, bt:bt + 1], xt[bt][:, ts(c, P)],
                                                   OP.mult, OP.add)
                nc.scalar.dma_start(out_d.ap()[ts(bt, P), :], outsb[:])


        for _rep in range(reps):
            kernel_body()

    nc.compile()
    return nc


def prep_inputs(x, n1_w, n1_b, fk1_c, fk1_b, n2_w, n2_b, fk2_c, fk2_b,
                ca_w1, ca_b1, ca_w2, ca_b2, sa_w, sa_b, D=2048, G=8, BPC=256):
    """Host-side repack of the full inputs into per-core in_maps."""
    DC = D // P
    B = np.asarray(x).shape[0]
    xs = np.ascontiguousarray(np.asarray(x, dtype=np.float32).reshape(B, D))

    def prep_w(coef, bias):
        w = np.asarray(coef, dtype=np.float32).transpose(3, 2, 0, 1) * WSC  # [G, I, 2, O]
        w = w.reshape(G, DC, P, 2, D)
        # bf16 part: g=1 chunks ordered (ic, cos/sin) then bias chunk
        wb = w[0].transpose(0, 2, 1, 3).reshape(2 * DC * P, D)  # [(ic s) p, D]
        ext = np.zeros((P, D), dtype=np.float32)
        ext[0] = np.asarray(bias, dtype=np.float32) * WSC
        wb = np.concatenate([wb, ext], axis=0).astype(ml_dtypes.bfloat16)
        # fp8 part: pairs ordered (g, ic); rows [pr*P + p], cols [s*D + d]
        wf = w[1:].transpose(0, 1, 2, 3, 4).reshape(G - 1, DC, P, 2 * D)
        wf = wf.reshape((G - 1) * DC * P, 2 * D)
        wf = np.clip(wf, -FP8_CLIP, FP8_CLIP).astype(ml_dtypes.float8_e4m3)
        return np.ascontiguousarray(wb), np.ascontiguousarray(wf)

    wb0, wf0 = prep_w(fk1_c, fk1_b)
    wb1, wf1 = prep_w(fk2_c, fk2_b)
    lnw = np.ascontiguousarray(np.stack([n1_w, n2_w]).astype(np.float32))
    lnb = np.ascontiguousarray(np.stack([n1_b, n2_b]).astype(np.float32))
    mw1 = np.ascontiguousarray(np.asarray(ca_w1, np.float32).T).astype(ml_dtypes.bfloat16)
    mw2 = np.ascontiguousarray(np.asarray(ca_w2, np.float32).T).astype(ml_dtypes.bfloat16)
    sw = np.asarray(sa_w, dtype=np.float32)
    sa3 = np.array([sw[0, 0, 3, 3], sw[0, 1, 3, 3], np.asarray(sa_b, np.float32)[0]],
                   dtype=np.float32)
    shared = {
        "wb0": wb0, "wf0": wf0, "wb1": wb1, "wf1": wf1,
        "lnw": lnw, "lnb": lnb,
        "mw1": mw1, "mw2": mw2,
        "mb1": np.ascontiguousarray(np.asarray(ca_b1, np.float32)),
        "mb2": np.ascontiguousarray(np.asarray(ca_b2, np.float32)),
        "sa3": sa3,
    }
    return [{**shared, "x": xs[i * BPC:(i + 1) * BPC]} for i in range(N_CORES)]


_PROGRAM = [None]


def kernel(**inputs) -> np.ndarray:
    if _PROGRAM[0] is None:
        _PROGRAM[0] = build_program(**FULL_CFG)
    nc = _PROGRAM[0]
    in_maps = prep_inputs(**inputs)
    res = run_bass_kernel_spmd(nc, in_maps, list(range(N_CORES)))
    out = np.concatenate([r["out"] for r in res.results], axis=0)
    B, D = out.shape
    return np.ascontiguousarray(out.reshape(B, 1, D).astype(np.float32))


# revision 14
# speedup vs baseline: 1.7496x; 1.7496x over previous
"""Trainium2 Bass kernel for nn_AttFKANBlock (2-layer Fourier-KAN block + CBAM tail).

Strategy: pure data-parallel over batch B=2048 across 8 NeuronCores (256 rows/core,
all parameters replicated). The whole computation is independent per batch row, so
no collectives are needed.

Per core, each FKAN layer out[b,o] = sum_{i,g} cos/sin((g+1)*h[b,i]) * coef[c,o,i,g]
is one dense GEMM with contraction K = 2*G*D (+1 bias chunk), in mixed precision:
  - harmonic g=1 (92% of output variance, coef ~ 1/g^2) runs in bf16,
  - harmonics g=2..8 run in fp8 e4m3 with MatmulPerfMode.DoubleRow (2 K-subtiles
    per instruction, 2x PE throughput, weights at 1 byte -> ~half the HBM traffic),
  - all weights are pre-scaled by 2^11 on the host so the fp8 blocks sit in e4m3's
    normal range; the common scale keeps every chunk in ONE psum accumulation group
    per (batch-tile, out-chunk) and is folded out at drain via act scale=2^-11.
Fourier features sin/cos(2pi*t) use fp32 magic-number range reduction (+/-1.5*2^23)
on the Vector engine and the ACT Sin LUT, batched over 8 input chunks per
instruction to amortize the ACT 352ns fixed overhead; fp8 feature tiles are written
directly by ACT as [128, ic, 2(cos/sin), 256b] so a DoubleRow lhsT slice is free.
The CBAM tail (channel-attention MLP + spatial attention) runs in [o-part, b-free]
layout; spatial attention's 7x7 conv on 1x1 spatial input reduces to its center tap.
"""
import numpy as np
import ml_dtypes
from contextlib import ExitStack

import concourse.tile as tile
from concourse import bacc, mybir
from concourse.bass import ts, ds
from concourse.bass_utils import run_bass_kernel_spmd
from concourse.masks import make_identity

f32 = mybir.dt.float32
bf16 = mybir.dt.bfloat16
fp8 = mybir.dt.float8e4
AF = mybir.ActivationFunctionType
OP = mybir.AluOpType
DR = mybir.MatmulPerfMode.DoubleRow

P = 128
PI = float(np.pi)
TWO_PI = float(2.0 * np.pi)
MAGIC = float(1.5 * 2**23)  # fp32 round-to-nearest-integer magic constant
WSC = float(2.0**11)        # weight pre-scale (host side), folded out at drain
FP8_CLIP = 240.0            # TRN e4m3 max normal

FULL_CFG = dict(D=2048, G=8, BPC=256, HID=128, EPS=1e-5)
N_CORES = 8


def build_program(D=2048, G=8, BPC=256, HID=128, EPS=1e-5, w_bufs=4, fs_bufs=4,
                  reps=1, diag='none', interleave=False):
    DC = D // P              # D chunks (16)
    NBT = BPC // P           # batch tiles per core (2)
    QB = DC // 4             # ic chunks per feature batch (4)
    NQ = DC // QB            # feature batches per harmonic (4)
    NG8 = G - 1              # fp8 harmonics (2..G)
    NPAIR = NG8 * DC         # fp8 weight pairs per layer (112)
    NBCH = 2 * DC + 1        # bf16 chunks per layer incl. bias (33)
    OCW = min(512, D)        # psum free width
    OC = D // OCW            # o-chunks (4)
    assert OC * NBT <= 8 and HID == P and D % OCW == 0
    BNW = min(512, D)        # bn_stats chunk width
    BNC = D // BNW

    nc = bacc.Bacc("TRN2", target_bir_lowering=False, debug=False, num_devices=N_CORES)

    x_d = nc.dram_tensor("x", [BPC, D], f32, kind="ExternalInput")
    wb_ds = [nc.dram_tensor(f"wb{l}", [NBCH * P, D], bf16, kind="ExternalInput")
             for l in range(2)]
    wf_ds = [nc.dram_tensor(f"wf{l}", [NPAIR * P, 2 * D], fp8, kind="ExternalInput")
             for l in range(2)]
    lnw_d = nc.dram_tensor("lnw", [2, D], f32, kind="ExternalInput")
    lnb_d = nc.dram_tensor("lnb", [2, D], f32, kind="ExternalInput")
    mw1_d = nc.dram_tensor("mw1", [D, HID], bf16, kind="ExternalInput")   # ca_w1.T
    mw2_d = nc.dram_tensor("mw2", [HID, D], bf16, kind="ExternalInput")   # ca_w2.T
    mb1_d = nc.dram_tensor("mb1", [HID], f32, kind="ExternalInput")       # ca_b1
    mb2_d = nc.dram_tensor("mb2", [D], f32, kind="ExternalInput")         # ca_b2
    sa3_d = nc.dram_tensor("sa3", [3], f32, kind="ExternalInput")         # w00, w01, sa_b
    out_d = nc.dram_tensor("out", [BPC, D], f32, kind="ExternalOutput")

    with tile.TileContext(nc) as tc, ExitStack() as ctx:
        pool = lambda name, bufs, **kw: ctx.enter_context(tc.tile_pool(name=name, bufs=bufs, **kw))
        p_const = pool("const", 1)
        p_x = pool("x", NBT + 1)
        p_r = pool("r", NBT)
        p_big = pool("big", 3)
        p_zt = pool("zt", 1)
        p_f8 = pool("f8", 2)      # fp8 feature tiles [P, QB, 2, BPC]
        p_fb = pool("fb", 2)      # bf16 feature tiles [P, QB, 2, BPC]
        p_fs = pool("fs", fs_bufs)  # range-reduction temps [P, QB, BPC] f32
        p_wb = pool("wb", w_bufs)
        p_wf = pool("wf", w_bufs + 4)
        p_vt = pool("vt", DC)
        p_vtb = pool("vtb", DC)
        p_ca = pool("ca", 2)
        p_sm = pool("sm", 2)
        p_ps = pool("ps", 8, space="PSUM")
        p_dram = pool("dscratch", 1, space="DRAM")

        _psn = [0]

        def psum_tile():
            _psn[0] += 1
            return p_ps.tile([P, OCW], f32, tag="ps", name=f"ps{_psn[0]}")

        # ---- constants / parameter staging ----
        ident = p_const.tile([P, P], f32)
        make_identity(nc, ident)
        featx = p_const.tile([P, BPC], bf16)   # bias feature chunk: row0 = 1, rest 0
        nc.vector.memset(featx[:], 0.0)
        nc.vector.memset(featx[0:1, :], 1.0)
        halfpi = p_const.tile([P, 1], f32)
        nc.vector.memset(halfpi[:], PI / 2)
        epsb = p_const.tile([P, 1], f32)
        nc.vector.memset(epsb[:], float(EPS))
        ones_col = p_const.tile([P, 1], f32)
        nc.vector.memset(ones_col[:], 1.0)

        lnw_sb = p_const.tile([P, 2, DC], f32)
        lnb_sb = p_const.tile([P, 2, DC], f32)
        with nc.allow_non_contiguous_dma(reason="small param staging"):
            nc.sync.dma_start(lnw_sb[:], lnw_d.ap().rearrange("l (c p) -> p l c", p=P))
            nc.sync.dma_start(lnb_sb[:], lnb_d.ap().rearrange("l (c p) -> p l c", p=P))
        # scale by 1/(2pi) so zt holds (z*w + b)/(2pi)
        nc.vector.tensor_scalar_mul(lnw_sb[:], lnw_sb[:], 1.0 / TWO_PI)
        nc.vector.tensor_scalar_mul(lnb_sb[:], lnb_sb[:], 1.0 / TWO_PI)

        mw1_sb = p_const.tile([P, DC, HID], bf16)
        nc.sync.dma_start(mw1_sb[:], mw1_d.ap().rearrange("(c p) h -> p c h", p=P))
        mw2_sb = p_const.tile([P, D], bf16)
        nc.sync.dma_start(mw2_sb[:], mw2_d.ap())
        mb1_sb = p_const.tile([P, 1], f32)
        with nc.allow_non_contiguous_dma(reason="small param staging"):
            nc.sync.dma_start(mb1_sb[:], mb1_d.ap().rearrange("(p a) -> p a", a=1))
            mb2x2 = p_const.tile([P, DC], f32)
            nc.sync.dma_start(mb2x2[:], mb2_d.ap().rearrange("(c p) -> p c", p=P))
        nc.vector.tensor_scalar_mul(mb2x2[:], mb2x2[:], 2.0)
        sa3_sb = p_const.tile([1, 3], f32)
        with nc.allow_non_contiguous_dma(reason="small param staging"):
            nc.sync.dma_start(sa3_sb[:], sa3_d.ap().rearrange("(a k) -> a k", a=1))

        def kernel_body():
            # ---- load x ----
            xt = []
            for bt in range(NBT):
                t = p_x.tile([P, D], f32)
                nc.scalar.dma_start(t[:], x_d.ap()[ts(bt, P), :])
                xt.append(t)

            # ---- helpers ----
            def ln_modify(src, dst):
                """dst = (src - mean)/sqrt(var+eps), rows of [P, D]. dst may alias src."""
                stats = p_sm.tile([P, BNC, 6], f32, tag="bnstats")
                for c in range(BNC):
                    nc.vector.bn_stats(stats[:, c], src[:, ts(c, BNW)])
                aggr = p_sm.tile([P, 2], f32, tag="bnaggr")
                nc.vector.bn_aggr(aggr[:], stats[:].rearrange("p a b -> p (a b)"))
                sd = p_sm.tile([P, 1], f32, tag="sd")
                nc.scalar.activation(sd[:], aggr[:, 1:2], AF.Sqrt, bias=epsb[:])
                rstd = p_sm.tile([P, 1], f32, tag="rstd")
                nc.vector.reciprocal(rstd[:], sd[:])
                nc.vector.tensor_scalar(dst[:], src[:], aggr[:, 0:1], rstd[:],
                                        OP.subtract, OP.mult)

            def transpose_zt(z_tiles, l):
                """z [b-part, D] tiles -> zt [P, DC, BPC] with (z*w+b)/(2pi) applied."""
                zt = p_zt.tile([P, DC, BPC], f32)
                for bt in range(NBT):
                    for c in range(DC):
                        pst = psum_tile()
                        nc.tensor.transpose(pst[:, :P], z_tiles[bt][:, ts(c, P)], ident)
                        nc.scalar.activation(zt[:, c, ds(bt * P, P)], pst[:, :P], AF.Identity,
                                             bias=lnb_sb[:, l, c:c + 1],
                                             scale=lnw_sb[:, l, c:c + 1])
                return zt

            def features_quarter(zt, g, q, out_dt, fpool):
                """Range-reduce zt[:, q*QB:(q+1)*QB, :] * g and emit a feature tile
                [P, QB, 2, BPC] (dim2: 0=cos, 1=sin) in out_dt."""
                X = zt[:, ds(q * QB, QB), :]
                if g != 1:
                    tt = p_fs.tile([P, QB, BPC], f32, tag="fs")
                    nc.vector.tensor_scalar_mul(tt[:], X, float(g))
                    src = tt[:]
                else:
                    src = X
                # nsp = src + MAGIC ; in-place -> dsin = (nsp - MAGIC) - src
                nsp = p_fs.tile([P, QB, BPC], f32, tag="fs")
                nc.vector.tensor_scalar_add(nsp[:], src, MAGIC)
                nc.vector.scalar_tensor_tensor(nsp[:], nsp[:], MAGIC, src,
                                               OP.subtract, OP.subtract)
                # ncp = (src + 0.25) + MAGIC ; in-place -> dcos = (ncp - MAGIC) - src
                ncp = p_fs.tile([P, QB, BPC], f32, tag="fs")
                nc.vector.tensor_scalar(ncp[:], src, 0.25, MAGIC, OP.add, OP.add)
                nc.vector.scalar_tensor_tensor(ncp[:], ncp[:], MAGIC, src,
                                               OP.subtract, OP.subtract)
                feat = fpool.tile([P, QB, 2, BPC], out_dt, tag="feat")
                nc.scalar.activation(feat[:, :, 0, :], ncp[:], AF.Sin, bias=halfpi[:],
                                     scale=-TWO_PI)
                nc.scalar.activation(feat[:, :, 1, :], nsp[:], AF.Sin, scale=-TWO_PI)
                return feat

            def fkan_matmul(wb_d, wf_d, zt):
                """Stream all weights once; accs[bt][oc] psum tiles [128b, OCW o].

                bf16 g=1 chunks and fp8 DoubleRow pairs are kept in separate
                phases: alternating PE matmul modes per chunk measured 1.7x
                slower on HW (mode/FWL toggling), despite the cost model
                preferring interleave. The bias chunk goes last (stop=True)."""
                accs = [[psum_tile() for _ in range(OC)] for _ in range(NBT)]
                first = [True]
                if diag == 'pe':
                    wt_b1 = p_wb.tile([P, D], bf16, tag="wb")
                    nc.gpsimd.memset(wt_b1[:], 0.25)
                    wt_f1 = p_wf.tile([P, 2, D], fp8, tag="wf")
                    nc.gpsimd.memset(wt_f1[:], 0.25)

                def emit_bf16(feat, icl, j, k):
                    if diag != 'pe':
                        wt = p_wb.tile([P, D], bf16, tag="wb")
                        nc.sync.dma_start(wt[:], wb_d.ap()[ts(k, P), :])
                    else:
                        wt = wt_b1
                    if diag != 'dma':
                        for bt in range(NBT):
                            lhsT = feat[:, icl, j, ts(bt, P)]
                            for oc in range(OC):
                                nc.tensor.matmul(accs[bt][oc][:], lhsT,
                                                 wt[:, ts(oc, OCW)],
                                                 start=first[0], stop=False)
                            first[0] = False

                def emit_fp8(feat, icl, pr):
                    if diag != 'pe':
                        wt = p_wf.tile([P, 2, D], fp8, tag="wf")
                        nc.sync.dma_start(wt[:], wf_d.ap()[ts(pr, P), :]
                                          .rearrange("p (j d) -> p j d", j=2))
                    else:
                        wt = wt_f1
                    if diag != 'dma':
                        for bt in range(NBT):
                            lhsT = feat[:, icl, :, ts(bt, P)]
                            for oc in range(OC):
                                nc.tensor.matmul(accs[bt][oc][:], lhsT,
                                                 wt[:, :, ts(oc, OCW)],
                                                 start=False, stop=False,
                                                 perf_mode=DR)

                if interleave:
                    for q in range(NQ):
                        featb = features_quarter(zt, 1, q, bf16, p_fb)
                        bq = [(icl, j) for icl in range(QB) for j in range(2)]
                        bqi = 0
                        icl, j = bq[bqi]
                        emit_bf16(featb, icl, j, (q * QB + icl) * 2 + j)
                        bqi += 1
                        for g in range(2, G + 1):
                            feat = features_quarter(zt, g, q, fp8, p_f8)
                            for icl in range(QB):
                                emit_fp8(feat, icl, (g - 2) * DC + q * QB + icl)
                            if bqi < len(bq):
                                icl, j = bq[bqi]
                                emit_bf16(featb, icl, j, (q * QB + icl) * 2 + j)
                                bqi += 1
                        while bqi < len(bq):
                            icl, j = bq[bqi]
                            emit_bf16(featb, icl, j, (q * QB + icl) * 2 + j)
                            bqi += 1
                else:
                    # phase 1: all bf16 g=1 chunks
                    for q in range(NQ):
                        featb = features_quarter(zt, 1, q, bf16, p_fb)
                        for icl in range(QB):
                            for j in range(2):
                                emit_bf16(featb, icl, j, (q * QB + icl) * 2 + j)
                    # phase 2: all fp8 pairs
                    for g in range(2, G + 1):
                        for q in range(NQ):
                            feat = features_quarter(zt, g, q, fp8, p_f8)
                            for icl in range(QB):
                                emit_fp8(feat, icl, (g - 2) * DC + q * QB + icl)

                # bias chunk last: feature row0 = 1.0, carries stop=True
                if diag != 'pe':
                    wt = p_wb.tile([P, D], bf16, tag="wb")
                    nc.sync.dma_start(wt[:], wb_d.ap()[ts(NBCH - 1, P), :])
                else:
                    wt = wt_b1
                if diag != 'dma':
                    for bt in range(NBT):
                        for oc in range(OC):
                            nc.tensor.matmul(accs[bt][oc][:], featx[:, ts(bt, P)],
                                             wt[:, ts(oc, OCW)], start=False, stop=True)
                return accs

            # ---- layer 1 ----
            z1 = []
            for bt in range(NBT):
                z = p_big.tile([P, D], f32, tag="big")
                ln_modify(xt[bt], z)
                z1.append(z)
            zt1 = transpose_zt(z1, 0)
            accs1 = fkan_matmul(wb_ds[0], wf_ds[0], zt1)

            # relu drain -> r (folds out the 2^11 weight scale)
            rt = []
            for bt in range(NBT):
                r = p_r.tile([P, D], f32)
                for oc in range(OC):
                    nc.scalar.activation(r[:, ts(oc, OCW)], accs1[bt][oc][:], AF.Relu,
                                         scale=1.0 / WSC)
                rt.append(r)

            # ---- layer 2 ----
            for bt in range(NBT):
                ln_modify(rt[bt], rt[bt])
            zt2 = transpose_zt(rt, 1)
            accs2 = fkan_matmul(wb_ds[1], wf_ds[1], zt2)

            # drain f2 in [b,o] layout
            f2 = []
            for bt in range(NBT):
                f = p_big.tile([P, D], f32, tag="big")
                for oc in range(OC):
                    nc.scalar.activation(f[:, ts(oc, OCW)], accs2[bt][oc][:], AF.Copy,
                                         scale=1.0 / WSC)
                f2.append(f)

            # ---- CBAM tail in [o-part, b-free] layout ----
            vt = [p_vt.tile([P, BPC], f32, tag="vt", name=f"vt{c}") for c in range(DC)]
            vtb = [p_vtb.tile([P, BPC], bf16, tag="vtb", name=f"vtb{c}") for c in range(DC)]
            for bt in range(NBT):
                for c in range(DC):
                    pst = psum_tile()
                    nc.tensor.transpose(pst[:, :P], f2[bt][:, ts(c, P)], ident)
                    nc.scalar.activation(vt[c][:, ts(bt, P)], pst[:, :P], AF.Copy)
                    nc.vector.tensor_copy(vtb[c][:, ts(bt, P)], pst[:, :P])

            # channel-attention MLP: ca = sigmoid(2*(W2 relu(W1 v + b1) + b2))
            psm = psum_tile()
            for c in range(DC):
                nc.tensor.matmul(psm[:, :BPC], mw1_sb[:, c], vtb[c][:],
                                 start=(c == 0), stop=(c == DC - 1))
            hbuf = p_sm.tile([P, BPC], bf16, tag="hbuf")
            nc.scalar.activation(hbuf[:], psm[:, :BPC], AF.Relu, bias=mb1_sb[:])
            # xc = v * ca (in place on vt), ca consumed as produced
            for c in range(DC):
                ps2 = psum_tile()
                nc.tensor.matmul(ps2[:, :BPC], mw2_sb[:, ts(c, P)], hbuf[:],
                                 start=True, stop=True)
                cac = p_ca.tile([P, BPC], bf16, tag="ca")
                nc.scalar.activation(cac[:], ps2[:, :BPC], AF.Sigmoid,
                                     bias=mb2x2[:, c:c + 1], scale=2.0)
                nc.vector.tensor_mul(vt[c][:], vt[c][:], cac[:])

            # spatial attention: sg = sigmoid(w00*mean_c + w01*max_c + sa_b)
            psmean = psum_tile()
            for c in range(DC):
                nc.tensor.matmul(psmean[:1, :BPC], ones_col[:, :1], vt[c][:],
                                 start=(c == 0), stop=(c == DC - 1))
            meanc = p_sm.tile([1, BPC], f32, tag="meanc")
            nc.vector.tensor_scalar_mul(meanc[:], psmean[:1, :BPC], 1.0 / D)
            mx = p_sm.tile([P, BPC], f32, tag="mx")
            nc.vector.tensor_copy(mx[:], vt[0][:])
            for c in range(1, DC):
                nc.vector.tensor_max(mx[:], mx[:], vt[c][:])
            # cross-partition max: PE-transpose each batch tile, reduce over free dim,
            # then round-trip through DRAM to land back in [1, BPC] row layout.
            maxd = p_dram.tile([1, BPC], f32)
            for bt in range(NBT):
                pst = psum_tile()
                nc.tensor.transpose(pst[:, :P], mx[:, ts(bt, P)], ident)
                maxb = p_sm.tile([P, 1], f32, tag="maxb")
                nc.vector.tensor_reduce(maxb[:], pst[:, :P], mybir.AxisListType.X, OP.max)
                with nc.allow_non_contiguous_dma(reason="tiny max round-trip"):
                    nc.scalar.dma_start(maxd[:, ds(bt * P, P)].rearrange("a p -> p a"), maxb[:])
            maxrow = p_sm.tile([1, BPC], f32, tag="maxrow")
            nc.scalar.dma_start(maxrow[:], maxd[:])
            t2 = p_sm.tile([1, BPC], f32, tag="t2")
            nc.vector.tensor_scalar_mul(t2[:], maxrow[:], sa3_sb[:, 1:2])
            sa_t = p_sm.tile([1, BPC], f32, tag="sat")
            nc.vector.scalar_tensor_tensor(sa_t[:], meanc[:], sa3_sb[:, 0:1], t2[:],
                                           OP.mult, OP.add)
            sg = p_sm.tile([1, BPC], f32, tag="sg")
            nc.scalar.activation(sg[:], sa_t[:], AF.Sigmoid, bias=sa3_sb[:, 2:3])

            # broadcast sg over partitions via DRAM round-trip -> [P, NBT] per-partition scalars
            sgd = p_dram.tile([1, BPC], f32)
            nc.scalar.dma_start(sgd[:], sg[:])
            sgpb = p_sm.tile([P, NBT], f32, tag="sgpb")
            with nc.allow_non_contiguous_dma(reason="tiny broadcast round-trip"):
                nc.scalar.dma_start(sgpb[:], sgd[:].rearrange("a (t p) -> p (a t)", p=P))

            # final: out = x + xc^T * sg
            for bt in range(NBT):
                outsb = p_big.tile([P, D], f32, tag="big")
                for c in range(DC):
                    pst = psum_tile()
                    nc.tensor.transpose(pst[:, :P], vt[c][:, ts(bt, P)], ident)
                    nc.vector.scalar_tensor_tensor(outsb[:, ts(c, P)], pst[:, :P],
                                                   sgpb[:, bt:bt + 1], xt[bt][:, ts(c, P)],
                                                   OP.mult, OP.add)
                nc.scalar.dma_start(out_d.ap()[ts(bt, P), :], outsb[:])


        for _rep in range(reps):
            kernel_body()

    nc.compile()
    return nc


def prep_inputs(x, n1_w, n1_b, fk1_c, fk1_b, n2_w, n2_b, fk2_c, fk2_b,
                ca_w1, ca_b1, ca_w2, ca_b2, sa_w, sa_b, D=2048, G=8, BPC=256):
    """Host-side repack of the full inputs into per-core in_maps."""
    DC = D // P
    B = np.asarray(x).shape[0]
    xs = np.ascontiguousarray(np.asarray(x, dtype=np.float32).reshape(B, D))

    def prep_w(coef, bias):
        w = np.asarray(coef, dtype=np.float32).transpose(3, 2, 0, 1) * WSC  # [G, I, 2, O]
        w = w.reshape(G, DC, P, 2, D)
        # bf16 part: g=1 chunks ordered (ic, cos/sin) then bias chunk
        wb = w[0].transpose(0, 2, 1, 3).reshape(2 * DC * P, D)  # [(ic s) p, D]
        ext = np.zeros((P, D), dtype=np.float32)
        ext[0] = np.asarray(bias, dtype=np.float32) * WSC
        wb = np.concatenate([wb, ext], axis=0).astype(ml_dtypes.bfloat16)
        # fp8 part: pairs ordered (g, ic); rows [pr*P + p], cols [s*D + d]
        wf = w[1:].transpose(0, 1, 2, 3, 4).reshape(G - 1, DC, P, 2 * D)
        wf = wf.reshape((G - 1) * DC * P, 2 * D)
        wf = np.clip(wf, -FP8_CLIP, FP8_CLIP).astype(ml_dtypes.float8_e4m3)
        return np.ascontiguousarray(wb), np.ascontiguousarray(wf)

    wb0, wf0 = prep_w(fk1_c, fk1_b)
    wb1, wf1 = prep_w(fk2_c, fk2_b)
    lnw = np.ascontiguousarray(np.stack([n1_w, n2_w]).astype(np.float32))
    lnb = np.ascontiguousarray(np.stack([n1_b, n2_b]).astype(np.float32))
    mw1 = np.ascontiguousarray(np.asarray(ca_w1, np.float32).T).astype(ml_dtypes.bfloat16)
    mw2 = np.ascontiguousarray(np.asarray(ca_w2, np.float32).T).astype(ml_dtypes.bfloat16)
    sw = np.asarray(sa_w, dtype=np.float32)
    sa3 = np.array([sw[0, 0, 3, 3], sw[0, 1, 3, 3], np.asarray(sa_b, np.float32)[0]],
                   dtype=np.float32)
    shared = {
        "wb0": wb0, "wf0": wf0, "wb1": wb1, "wf1": wf1,
        "lnw": lnw, "lnb": lnb,
        "mw1": mw1, "mw2": mw2,
        "mb1": np.ascontiguousarray(np.asarray(ca_b1, np.float32)),
        "mb2": np.ascontiguousarray(np.asarray(ca_b2, np.float32)),
        "sa3": sa3,
    }
    return [{**shared, "x": xs[i * BPC:(i + 1) * BPC]} for i in range(N_CORES)]


_PROGRAM = [None]


def kernel(**inputs) -> np.ndarray:
    if _PROGRAM[0] is None:
        _PROGRAM[0] = build_program(**FULL_CFG)
    nc = _PROGRAM[0]
    in_maps = prep_inputs(**inputs)
    res = run_bass_kernel_spmd(nc, in_maps, list(range(N_CORES)))
    out = np.concatenate([r["out"] for r in res.results], axis=0)
    B, D = out.shape
    return np.ascontiguousarray(out.reshape(B, 1, D).astype(np.float32))


# revision 16
# speedup vs baseline: 2.3006x; 1.3149x over previous
"""Trainium2 Bass kernel for nn_AttFKANBlock (2-layer Fourier-KAN block + CBAM tail).

Strategy: pure data-parallel over batch B=2048 across 8 NeuronCores (256 rows/core,
all parameters replicated). The whole computation is independent per batch row, so
no collectives are needed.

Per core, each FKAN layer out[b,o] = sum_{i,g} cos/sin((g+1)*h[b,i]) * coef[c,o,i,g]
is one dense GEMM with contraction K = 2*G*D (+1 bias chunk), in mixed precision:
  - harmonic g=1 (92% of output variance, coef ~ 1/g^2) runs in bf16,
  - harmonics g=2..8 run in fp8 e4m3 with MatmulPerfMode.DoubleRow (2 K-subtiles
    per instruction, 2x PE throughput, weights at 1 byte -> ~half the HBM traffic),
  - all weights are pre-scaled by 2^11 on the host so the fp8 blocks sit in e4m3's
    normal range; the common scale keeps every chunk in ONE psum accumulation group
    per (batch-tile, out-chunk) and is folded out at drain via act scale=2^-11.
Fourier features sin/cos(2pi*t) use fp32 magic-number range reduction (+/-1.5*2^23)
on the Vector engine and the ACT Sin LUT, batched over 8 input chunks per
instruction to amortize the ACT 352ns fixed overhead; fp8 feature tiles are written
directly by ACT as [128, ic, 2(cos/sin), 256b] so a DoubleRow lhsT slice is free.
The CBAM tail (channel-attention MLP + spatial attention) runs in [o-part, b-free]
layout; spatial attention's 7x7 conv on 1x1 spatial input reduces to its center tap.
"""
import numpy as np
import ml_dtypes
from contextlib import ExitStack

import concourse.tile as tile
from concourse import bacc, mybir
from concourse.bass import ts, ds
from concourse.bass_utils import run_bass_kernel_spmd
from concourse.masks import make_identity

f32 = mybir.dt.float32
bf16 = mybir.dt.bfloat16
fp16 = mybir.dt.float16
fp8 = mybir.dt.float8e4
AF = mybir.ActivationFunctionType
OP = mybir.AluOpType
DR = mybir.MatmulPerfMode.DoubleRow

P = 128
PI = float(np.pi)
TWO_PI = float(2.0 * np.pi)
MAGIC = float(1.5 * 2**23)  # fp32 round-to-nearest-integer magic constant
WSC = float(2.0**11)        # weight pre-scale (host side), folded out at drain
FP8_CLIP = 240.0            # TRN e4m3 max normal

FULL_CFG = dict(D=2048, G=8, GF=8, BPC=256, HID=128, EPS=1e-5)
N_CORES = 8


def build_program(D=2048, G=8, GF=8, BPC=256, HID=128, EPS=1e-5, w_bufs=4, fs_bufs=4,
                  reps=1, diag='none', interleave=False):
    DC = D // P              # D chunks (16)
    NBT = BPC // P           # batch tiles per core (2)
    QB = DC // 4             # ic chunks per feature batch (4)
    NQ = DC // QB            # feature batches per harmonic (4)
    NG8 = GF - 1             # fp8 harmonics (2..GF; GF<G drops top harmonics)
    NPAIR = NG8 * DC         # fp8 weight pairs per layer
    NBCH = 2 * DC + 1        # bf16 chunks per layer incl. bias (33)
    OCW = min(512, D)        # psum free width
    OC = D // OCW            # o-chunks (4)
    assert OC * NBT <= 8 and HID == P and D % OCW == 0
    BNW = min(512, D)        # bn_stats chunk width
    BNC = D // BNW

    nc = bacc.Bacc("TRN2", target_bir_lowering=False, debug=False, num_devices=N_CORES)

    x_d = nc.dram_tensor("x", [BPC, D], f32, kind="ExternalInput")
    wb_ds = [nc.dram_tensor(f"wb{l}", [NBCH * P, D], fp16, kind="ExternalInput")
             for l in range(2)]
    wf_ds = [nc.dram_tensor(f"wf{l}", [NPAIR * P, 2 * D], fp8, kind="ExternalInput")
             for l in range(2)]
    lnw_d = nc.dram_tensor("lnw", [2, D], f32, kind="ExternalInput")
    lnb_d = nc.dram_tensor("lnb", [2, D], f32, kind="ExternalInput")
    mw1_d = nc.dram_tensor("mw1", [D, HID], bf16, kind="ExternalInput")   # ca_w1.T
    mw2_d = nc.dram_tensor("mw2", [HID, D], bf16, kind="ExternalInput")   # ca_w2.T
    mb1_d = nc.dram_tensor("mb1", [HID], f32, kind="ExternalInput")       # ca_b1
    mb2_d = nc.dram_tensor("mb2", [D], f32, kind="ExternalInput")         # ca_b2
    sa3_d = nc.dram_tensor("sa3", [3], f32, kind="ExternalInput")         # w00, w01, sa_b
    out_d = nc.dram_tensor("out", [BPC, D], f32, kind="ExternalOutput")

    with tile.TileContext(nc) as tc, ExitStack() as ctx:
        pool = lambda name, bufs, **kw: ctx.enter_context(tc.tile_pool(name=name, bufs=bufs, **kw))
        p_const = pool("const", 1)
        p_x = pool("x", NBT + 1)
        p_r = pool("r", NBT)
        p_big = pool("big", 3)
        p_zt = pool("zt", 1)
        p_f8 = pool("f8", 2)      # fp8 feature tiles [P, QB, 2, BPC]
        p_fb = pool("fb", 2)      # bf16 feature tiles [P, QB, 2, BPC]
        p_fs = pool("fs", fs_bufs)  # range-reduction temps [P, QB, BPC] f32
        p_wb = pool("wb", w_bufs)
        p_wf = pool("wf", w_bufs + 4)
        p_vt = pool("vt", DC)
        p_vtb = pool("vtb", DC)
        p_ca = pool("ca", 2)
        p_sm = pool("sm", 2)
        p_ps = pool("ps", 8, space="PSUM")
        p_dram = pool("dscratch", 1, space="DRAM")

        _psn = [0]

        def psum_tile():
            _psn[0] += 1
            return p_ps.tile([P, OCW], f32, tag="ps", name=f"ps{_psn[0]}")

        # ---- constants / parameter staging ----
        ident = p_const.tile([P, P], f32)
        make_identity(nc, ident)
        featx = p_const.tile([P, BPC], fp16)   # bias feature chunk: row0 = 1, rest 0
        nc.vector.memset(featx[:], 0.0)
        nc.vector.memset(featx[0:1, :], 1.0)
        halfpi = p_const.tile([P, 1], f32)
        nc.vector.memset(halfpi[:], PI / 2)
        epsb = p_const.tile([P, 1], f32)
        nc.vector.memset(epsb[:], float(EPS))
        ones_col = p_const.tile([P, 1], f32)
        nc.vector.memset(ones_col[:], 1.0)

        lnw_sb = p_const.tile([P, 2, DC], f32)
        lnb_sb = p_const.tile([P, 2, DC], f32)
        with nc.allow_non_contiguous_dma(reason="small param staging"):
            nc.sync.dma_start(lnw_sb[:], lnw_d.ap().rearrange("l (c p) -> p l c", p=P))
            nc.sync.dma_start(lnb_sb[:], lnb_d.ap().rearrange("l (c p) -> p l c", p=P))
        # scale by 1/(2pi) so zt holds (z*w + b)/(2pi)
        nc.vector.tensor_scalar_mul(lnw_sb[:], lnw_sb[:], 1.0 / TWO_PI)
        nc.vector.tensor_scalar_mul(lnb_sb[:], lnb_sb[:], 1.0 / TWO_PI)

        mw1_sb = p_const.tile([P, DC, HID], bf16)
        nc.sync.dma_start(mw1_sb[:], mw1_d.ap().rearrange("(c p) h -> p c h", p=P))
        mw2_sb = p_const.tile([P, D], bf16)
        nc.sync.dma_start(mw2_sb[:], mw2_d.ap())
        mb1_sb = p_const.tile([P, 1], f32)
        with nc.allow_non_contiguous_dma(reason="small param staging"):
            nc.sync.dma_start(mb1_sb[:], mb1_d.ap().rearrange("(p a) -> p a", a=1))
            mb2x2 = p_const.tile([P, DC], f32)
            nc.sync.dma_start(mb2x2[:], mb2_d.ap().rearrange("(c p) -> p c", p=P))
        nc.vector.tensor_scalar_mul(mb2x2[:], mb2x2[:], 2.0)
        sa3_sb = p_const.tile([1, 3], f32)
        with nc.allow_non_contiguous_dma(reason="small param staging"):
            nc.sync.dma_start(sa3_sb[:], sa3_d.ap().rearrange("(a k) -> a k", a=1))

        def kernel_body():
            # ---- load x ----
            xt = []
            for bt in range(NBT):
                t = p_x.tile([P, D], f32)
                nc.scalar.dma_start(t[:], x_d.ap()[ts(bt, P), :])
                xt.append(t)

            # ---- helpers ----
            def ln_modify(src, dst):
                """dst = (src - mean)/sqrt(var+eps), rows of [P, D]. dst may alias src."""
                stats = p_sm.tile([P, BNC, 6], f32, tag="bnstats")
                for c in range(BNC):
                    nc.vector.bn_stats(stats[:, c], src[:, ts(c, BNW)])
                aggr = p_sm.tile([P, 2], f32, tag="bnaggr")
                nc.vector.bn_aggr(aggr[:], stats[:].rearrange("p a b -> p (a b)"))
                sd = p_sm.tile([P, 1], f32, tag="sd")
                nc.scalar.activation(sd[:], aggr[:, 1:2], AF.Sqrt, bias=epsb[:])
                rstd = p_sm.tile([P, 1], f32, tag="rstd")
                nc.vector.reciprocal(rstd[:], sd[:])
                nc.vector.tensor_scalar(dst[:], src[:], aggr[:, 0:1], rstd[:],
                                        OP.subtract, OP.mult)

            def transpose_zt(z_tiles, l):
                """z [b-part, D] tiles -> zt [P, DC, BPC] with (z*w+b)/(2pi) applied."""
                zt = p_zt.tile([P, DC, BPC], f32)
                for bt in range(NBT):
                    for c in range(DC):
                        pst = psum_tile()
                        nc.tensor.transpose(pst[:, :P], z_tiles[bt][:, ts(c, P)], ident)
                        nc.scalar.activation(zt[:, c, ds(bt * P, P)], pst[:, :P], AF.Identity,
                                             bias=lnb_sb[:, l, c:c + 1],
                                             scale=lnw_sb[:, l, c:c + 1])
                return zt

            def features_quarter(zt, g, q, out_dt, fpool):
                """Range-reduce zt[:, q*QB:(q+1)*QB, :] * g and emit a feature tile
                [P, QB, 2, BPC] (dim2: 0=cos, 1=sin) in out_dt."""
                X = zt[:, ds(q * QB, QB), :]
                if g != 1:
                    tt = p_fs.tile([P, QB, BPC], f32, tag="fs")
                    nc.vector.tensor_scalar_mul(tt[:], X, float(g))
                    src = tt[:]
                else:
                    src = X
                # nsp = src + MAGIC ; in-place -> dsin = (nsp - MAGIC) - src
                nsp = p_fs.tile([P, QB, BPC], f32, tag="fs")
                nc.vector.tensor_scalar_add(nsp[:], src, MAGIC)
                nc.vector.scalar_tensor_tensor(nsp[:], nsp[:], MAGIC, src,
                                               OP.subtract, OP.subtract)
                # ncp = (src + 0.25) + MAGIC ; in-place -> dcos = (ncp - MAGIC) - src
                ncp = p_fs.tile([P, QB, BPC], f32, tag="fs")
                nc.vector.tensor_scalar(ncp[:], src, 0.25, MAGIC, OP.add, OP.add)
                nc.vector.scalar_tensor_tensor(ncp[:], ncp[:], MAGIC, src,
                                               OP.subtract, OP.subtract)
                feat = fpool.tile([P, QB, 2, BPC], out_dt, tag="feat")
                nc.scalar.activation(feat[:, :, 0, :], ncp[:], AF.Sin, bias=halfpi[:],
                                     scale=-TWO_PI)
                nc.scalar.activation(feat[:, :, 1, :], nsp[:], AF.Sin, scale=-TWO_PI)
                return feat

            def fkan_matmul(wb_d, wf_d, zt):
                """Stream all weights once; accs[bt][oc] psum tiles [128b, OCW o].

                bf16 g=1 chunks and fp8 DoubleRow pairs are kept in separate
                phases: alternating PE matmul modes per chunk measured 1.7x
                slower on HW (mode/FWL toggling), despite the cost model
                preferring interleave. The bias chunk goes last (stop=True)."""
                accs = [[psum_tile() for _ in range(OC)] for _ in range(NBT)]
                first = [True]
                if diag == 'pe':
                    wt_b1 = p_wb.tile([P, D], fp16, tag="wb")
                    nc.gpsimd.memset(wt_b1[:], 0.25)
                    wt_f1 = p_wf.tile([P, 2, D], fp8, tag="wf")
                    nc.gpsimd.memset(wt_f1[:], 0.25)

                def emit_bf16(feat, icl, j, k):
                    if diag != 'pe':
                        wt = p_wb.tile([P, D], fp16, tag="wb")
                        nc.sync.dma_start(wt[:], wb_d.ap()[ts(k, P), :])
                    else:
                        wt = wt_b1
                    if diag != 'dma':
                        for bt in range(NBT):
                            lhsT = feat[:, icl, j, ts(bt, P)]
                            for oc in range(OC):
                                nc.tensor.matmul(accs[bt][oc][:], lhsT,
                                                 wt[:, ts(oc, OCW)],
                                                 start=first[0], stop=False)
                            first[0] = False

                def emit_fp8(feat, icl, pr):
                    if diag != 'pe':
                        wt = p_wf.tile([P, 2, D], fp8, tag="wf")
                        nc.sync.dma_start(wt[:], wf_d.ap()[ts(pr, P), :]
                                          .rearrange("p (j d) -> p j d", j=2))
                    else:
                        wt = wt_f1
                    if diag != 'dma':
                        for bt in range(NBT):
                            lhsT = feat[:, icl, :, ts(bt, P)]
                            for oc in range(OC):
                                nc.tensor.matmul(accs[bt][oc][:], lhsT,
                                                 wt[:, :, ts(oc, OCW)],
                                                 start=False, stop=False,
                                                 perf_mode=DR)

                if interleave:
                    for q in range(NQ):
                        featb = features_quarter(zt, 1, q, bf16, p_fb)
                        bq = [(icl, j) for icl in range(QB) for j in range(2)]
                        bqi = 0
                        icl, j = bq[bqi]
                        emit_bf16(featb, icl, j, (q * QB + icl) * 2 + j)
                        bqi += 1
                        for g in range(2, G + 1):
                            feat = features_quarter(zt, g, q, fp8, p_f8)
                            for icl in range(QB):
                                emit_fp8(feat, icl, (g - 2) * DC + q * QB + icl)
                            if bqi < len(bq):
                                icl, j = bq[bqi]
                                emit_bf16(featb, icl, j, (q * QB + icl) * 2 + j)
                                bqi += 1
                        while bqi < len(bq):
                            icl, j = bq[bqi]
                            emit_bf16(featb, icl, j, (q * QB + icl) * 2 + j)
                            bqi += 1
                else:
                    # phase 1: all bf16 g=1 chunks
                    for q in range(NQ):
                        featb = features_quarter(zt, 1, q, fp16, p_fb)
                        for icl in range(QB):
                            for j in range(2):
                                emit_bf16(featb, icl, j, (q * QB + icl) * 2 + j)
                    # phase 2: all fp8 pairs
                    for g in range(2, G + 1):
                        for q in range(NQ):
                            feat = features_quarter(zt, g, q, fp8, p_f8)
                            for icl in range(QB):
                                emit_fp8(feat, icl, (g - 2) * DC + q * QB + icl)

                # bias chunk last: feature row0 = 1.0, carries stop=True
                if diag != 'pe':
                    wt = p_wb.tile([P, D], fp16, tag="wb")
                    nc.sync.dma_start(wt[:], wb_d.ap()[ts(NBCH - 1, P), :])
                else:
                    wt = wt_b1
                if diag != 'dma':
                    for bt in range(NBT):
                        for oc in range(OC):
                            nc.tensor.matmul(accs[bt][oc][:], featx[:, ts(bt, P)],
                                             wt[:, ts(oc, OCW)], start=False, stop=True)
                else:
                    # psum must be written before the drain reads it
                    for bt in range(NBT):
                        for oc in range(OC):
                            nc.tensor.matmul(accs[bt][oc][:], featx[:, ts(bt, P)],
                                             wt[:, ts(oc, OCW)], start=True, stop=True)
                return accs

            # ---- layer 1 ----
            z1 = []
            for bt in range(NBT):
                z = p_big.tile([P, D], f32, tag="big")
                ln_modify(xt[bt], z)
                z1.append(z)
            zt1 = transpose_zt(z1, 0)
            accs1 = fkan_matmul(wb_ds[0], wf_ds[0], zt1)

            # relu drain -> r (folds out the 2^11 weight scale)
            rt = []
            for bt in range(NBT):
                r = p_r.tile([P, D], f32)
                for oc in range(OC):
                    nc.scalar.activation(r[:, ts(oc, OCW)], accs1[bt][oc][:], AF.Relu,
                                         scale=1.0 / WSC)
                rt.append(r)

            # ---- layer 2 ----
            for bt in range(NBT):
                ln_modify(rt[bt], rt[bt])
            zt2 = transpose_zt(rt, 1)
            accs2 = fkan_matmul(wb_ds[1], wf_ds[1], zt2)

            # drain f2 in [b,o] layout
            f2 = []
            for bt in range(NBT):
                f = p_big.tile([P, D], f32, tag="big")
                for oc in range(OC):
                    nc.scalar.activation(f[:, ts(oc, OCW)], accs2[bt][oc][:], AF.Copy,
                                         scale=1.0 / WSC)
                f2.append(f)

            # ---- CBAM tail in [o-part, b-free] layout ----
            vt = [p_vt.tile([P, BPC], f32, tag="vt", name=f"vt{c}") for c in range(DC)]
            vtb = [p_vtb.tile([P, BPC], bf16, tag="vtb", name=f"vtb{c}") for c in range(DC)]
            for bt in range(NBT):
                for c in range(DC):
                    pst = psum_tile()
                    nc.tensor.transpose(pst[:, :P], f2[bt][:, ts(c, P)], ident)
                    nc.scalar.activation(vt[c][:, ts(bt, P)], pst[:, :P], AF.Copy)
                    nc.vector.tensor_copy(vtb[c][:, ts(bt, P)], pst[:, :P])

            # channel-attention MLP: ca = sigmoid(2*(W2 relu(W1 v + b1) + b2))
            psm = psum_tile()
            for c in range(DC):
                nc.tensor.matmul(psm[:, :BPC], mw1_sb[:, c], vtb[c][:],
                                 start=(c == 0), stop=(c == DC - 1))
            hbuf = p_sm.tile([P, BPC], bf16, tag="hbuf")
            nc.scalar.activation(hbuf[:], psm[:, :BPC], AF.Relu, bias=mb1_sb[:])
            # xc = v * ca (in place on vt), ca consumed as produced
            for c in range(DC):
                ps2 = psum_tile()
                nc.tensor.matmul(ps2[:, :BPC], mw2_sb[:, ts(c, P)], hbuf[:],
                                 start=True, stop=True)
                cac = p_ca.tile([P, BPC], bf16, tag="ca")
                nc.scalar.activation(cac[:], ps2[:, :BPC], AF.Sigmoid,
                                     bias=mb2x2[:, c:c + 1], scale=2.0)
                nc.vector.tensor_mul(vt[c][:], vt[c][:], cac[:])

            # spatial attention: sg = sigmoid(w00*mean_c + w01*max_c + sa_b)
            psmean = psum_tile()
            for c in range(DC):
                nc.tensor.matmul(psmean[:1, :BPC], ones_col[:, :1], vt[c][:],
                                 start=(c == 0), stop=(c == DC - 1))
            meanc = p_sm.tile([1, BPC], f32, tag="meanc")
            nc.vector.tensor_scalar_mul(meanc[:], psmean[:1, :BPC], 1.0 / D)
            mx = p_sm.tile([P, BPC], f32, tag="mx")
            nc.vector.tensor_copy(mx[:], vt[0][:])
            for c in range(1, DC):
                nc.vector.tensor_max(mx[:], mx[:], vt[c][:])
            # cross-partition max: PE-transpose each batch tile, reduce over free dim,
            # then round-trip through DRAM to land back in [1, BPC] row layout.
            maxd = p_dram.tile([1, BPC], f32)
            for bt in range(NBT):
                pst = psum_tile()
                nc.tensor.transpose(pst[:, :P], mx[:, ts(bt, P)], ident)
                maxb = p_sm.tile([P, 1], f32, tag="maxb")
                nc.vector.tensor_reduce(maxb[:], pst[:, :P], mybir.AxisListType.X, OP.max)
                with nc.allow_non_contiguous_dma(reason="tiny max round-trip"):
                    nc.scalar.dma_start(maxd[:, ds(bt * P, P)].rearrange("a p -> p a"), maxb[:])
            maxrow = p_sm.tile([1, BPC], f32, tag="maxrow")
            nc.scalar.dma_start(maxrow[:], maxd[:])
            t2 = p_sm.tile([1, BPC], f32, tag="t2")
            nc.vector.tensor_scalar_mul(t2[:], maxrow[:], sa3_sb[:, 1:2])
            sa_t = p_sm.tile([1, BPC], f32, tag="sat")
            nc.vector.scalar_tensor_tensor(sa_t[:], meanc[:], sa3_sb[:, 0:1], t2[:],
                                           OP.mult, OP.add)
            sg = p_sm.tile([1, BPC], f32, tag="sg")
            nc.scalar.activation(sg[:], sa_t[:], AF.Sigmoid, bias=sa3_sb[:, 2:3])

            # broadcast sg over partitions via DRAM round-trip -> [P, NBT] per-partition scalars
            sgd = p_dram.tile([1, BPC], f32)
            nc.scalar.dma_start(sgd[:], sg[:])
            sgpb = p_sm.tile([P, NBT], f32, tag="sgpb")
            with nc.allow_non_contiguous_dma(reason="tiny broadcast round-trip"):
                nc.scalar.dma_start(sgpb[:], sgd[:].rearrange("a (t p) -> p (a t)", p=P))

            # final: out = x + xc^T * sg
            for bt in range(NBT):
                outsb = p_big.tile([P, D], f32, tag="big")
                for c in range(DC):
                    pst = psum_tile()
                    nc.tensor.transpose(pst[:, :P], vt[c][:, ts(bt, P)], ident)
                    nc.vector.scalar_tensor_tensor(outsb[:, ts(c, P)], pst[:, :P],
                                                   sgpb[:, bt:bt + 1], xt[bt][:, ts(c, P)],
                                                   OP.mult, OP.add)
                nc.scalar.dma_start(out_d.ap()[ts(bt, P), :], outsb[:])


        for _rep in range(reps):
            kernel_body()

    nc.compile()
    return nc


def prep_inputs(x, n1_w, n1_b, fk1_c, fk1_b, n2_w, n2_b, fk2_c, fk2_b,
                ca_w1, ca_b1, ca_w2, ca_b2, sa_w, sa_b, D=2048, G=8, GF=None, BPC=256):
    if GF is None:
        GF = FULL_CFG["GF"]
    """Host-side repack of the full inputs into per-core in_maps."""
    DC = D // P
    B = np.asarray(x).shape[0]
    xs = np.ascontiguousarray(np.asarray(x, dtype=np.float32).reshape(B, D))

    def prep_w(coef, bias):
        w = np.asarray(coef, dtype=np.float32).transpose(3, 2, 0, 1) * WSC  # [G, I, 2, O]
        w = w.reshape(G, DC, P, 2, D)
        # bf16 part: g=1 chunks ordered (ic, cos/sin) then bias chunk
        wb = w[0].transpose(0, 2, 1, 3).reshape(2 * DC * P, D)  # [(ic s) p, D]
        ext = np.zeros((P, D), dtype=np.float32)
        ext[0] = np.asarray(bias, dtype=np.float32) * WSC
        wb = np.concatenate([wb, ext], axis=0).astype(np.float16)
        # fp8 part: pairs ordered (g, ic); rows [pr*P + p], cols [s*D + d]
        wf = w[1:GF].reshape(GF - 1, DC, P, 2 * D)
        wf = wf.reshape((GF - 1) * DC * P, 2 * D)
        wf = np.clip(wf, -FP8_CLIP, FP8_CLIP).astype(ml_dtypes.float8_e4m3)
        return np.ascontiguousarray(wb), np.ascontiguousarray(wf)

    wb0, wf0 = prep_w(fk1_c, fk1_b)
    wb1, wf1 = prep_w(fk2_c, fk2_b)
    lnw = np.ascontiguousarray(np.stack([n1_w, n2_w]).astype(np.float32))
    lnb = np.ascontiguousarray(np.stack([n1_b, n2_b]).astype(np.float32))
    mw1 = np.ascontiguousarray(np.asarray(ca_w1, np.float32).T).astype(ml_dtypes.bfloat16)
    mw2 = np.ascontiguousarray(np.asarray(ca_w2, np.float32).T).astype(ml_dtypes.bfloat16)
    sw = np.asarray(sa_w, dtype=np.float32)
    sa3 = np.array([sw[0, 0, 3, 3], sw[0, 1, 3, 3], np.asarray(sa_b, np.float32)[0]],
                   dtype=np.float32)
    shared = {
        "wb0": wb0, "wf0": wf0, "wb1": wb1, "wf1": wf1,
        "lnw": lnw, "lnb": lnb,
        "mw1": mw1, "mw2": mw2,
        "mb1": np.ascontiguousarray(np.asarray(ca_b1, np.float32)),
        "mb2": np.ascontiguousarray(np.asarray(ca_b2, np.float32)),
        "sa3": sa3,
    }
    return [{**shared, "x": xs[i * BPC:(i + 1) * BPC]} for i in range(N_CORES)]


_PROGRAM = [None]


def kernel(**inputs) -> np.ndarray:
    if _PROGRAM[0] is None:
        _PROGRAM[0] = build_program(**FULL_CFG)
    nc = _PROGRAM[0]
    in_maps = prep_inputs(**inputs)
    res = run_bass_kernel_spmd(nc, in_maps, list(range(N_CORES)))
    out = np.concatenate([r["out"] for r in res.results], axis=0)
    B, D = out.shape
    return np.ascontiguousarray(out.reshape(B, 1, D).astype(np.float32))
